# revision 4
# baseline (speedup 1.0000x reference)
"""MoE layer (top-2 of 8 experts, d=1024, d_ff=4096) on 8 TRN2 NeuronCores.

Strategy: d_ff-parallel (tensor-parallel over the FFN hidden dim). The
router / top-k / softmax-gate computation is tiny (0.05% of FLOPs) and runs
on host in numpy. Tokens are grouped by expert into a single padded stream of
R rows (each expert segment padded to a multiple of 128). Every core
processes ALL R rows but only a 512-wide slice of d_ff:

    y_c = gate * (swish(x @ W1[e][:, c*512:(c+1)*512]) @ W2[e][c*512:(c+1)*512, :])

so per-core work is exactly 1/8 of the routed FLOPs with zero load
imbalance. The host sums the 8 partial outputs and combines the two expert
contributions per token. All matmuls run in bf16 with fp32 PSUM
accumulation; partial outputs return as bf16 (summed in f32 on host).

All device inputs are host-prearranged so every DMA is per-partition
contiguous (x is pre-tiled into [NTILES, 128, 8, 512] blocks).
"""

import math

import numpy as np
import ml_dtypes

D_MODEL, D_FF, N_EXPERTS, TOP_K = 1024, 4096, 8, 2
N_CORES = 8
P = 128
TN = 512  # token tile (free dim of matmul 1)
DS = D_FF // N_CORES  # 512: per-core d_ff slice
KO = D_MODEL // P  # 8 contraction tiles for matmul 1
MS = DS // P  # 4 d_ff subtiles per core
XPRE = 4  # x tiles prefetched ahead of the weight bulk

_KERNEL_CACHE: dict[tuple, object] = {}


def _token_tiles(seg_sizes):
    """(expert, start, size) tiles, sizes <=TN, multiples of 128, not
    crossing expert-segment boundaries."""
    tiles = []
    t0 = 0
    for e, seg in enumerate(seg_sizes):
        off = 0
        while off < seg:
            tn = min(TN, seg - off)
            tiles.append((e, t0 + off, tn))
            off += tn
        t0 += seg
    return tiles


def _build_device_kernel(seg_sizes: tuple[int, ...]):
    """Per-core program over the padded token stream.

    Inputs (per core):
      x  [NTILES, 128, 8, 512] bf16  pre-tiled x^T: x[i, p, ko, n] =
                                     xf[tok(tile_i, n), ko*128+p]
      w1 [128, 8, 8, 512]   bf16  w1[p, e, ko, f] = W1[e, ko*128+p, c*512+f]
      w2 [128, 8, 4, 1024]  bf16  w2[p, e, m, n] = W2[e, c*512+m*128+p, n]
      g  [128, R//128]      f32   g[p, a] = gate[a*128+p]
    Output:
      y  [128, R//128, 1024] bf16 partial FFN output (gate-scaled)
    """
    import concourse.mybir as mybir
    import concourse.tile as tile
    from concourse import bacc

    dt = mybir.dt
    R = sum(seg_sizes)
    NT = R // P
    tiles = _token_tiles(seg_sizes)
    NTILES = len(tiles)

    nc = bacc.Bacc("TRN2", target_bir_lowering=False, debug=False)

    x = nc.dram_tensor("x", [NTILES, P, KO, TN], dt.bfloat16, kind="ExternalInput")
    w1 = nc.dram_tensor("w1", [P, N_EXPERTS, KO, DS], dt.bfloat16, kind="ExternalInput")
    w2 = nc.dram_tensor("w2", [P, N_EXPERTS, MS, D_MODEL], dt.bfloat16, kind="ExternalInput")
    g = nc.dram_tensor("g", [P, NT], dt.float32, kind="ExternalInput")
    y = nc.dram_tensor("y", [P, NT, D_MODEL], dt.bfloat16, kind="ExternalOutput")

    first_e = tiles[0][0]
    expert_order = [first_e] + [e for e in range(N_EXPERTS) if e != first_e]

    with tile.TileContext(nc) as tc:
        with (
            tc.tile_pool(name="wpool", bufs=1) as wpool,
            tc.tile_pool(name="gpool", bufs=1) as gpool,
            tc.tile_pool(name="warmp", bufs=1) as warmp,
            tc.tile_pool(name="xpool", bufs=XPRE) as xpool,
            tc.tile_pool(name="hpool", bufs=2) as hpool,
            tc.tile_pool(name="ypool", bufs=2) as ypool,
            tc.tile_pool(name="ps1", bufs=3, space="PSUM") as ps1,
            tc.tile_pool(name="ps2", bufs=4, space="PSUM") as ps2,
            tc.tile_pool(name="psw", bufs=1, space="PSUM") as psw,
        ):
            w1_sb = wpool.tile([P, N_EXPERTS, KO, DS], dt.bfloat16)
            w2_sb = wpool.tile([P, N_EXPERTS, MS, D_MODEL], dt.bfloat16)
            g_sb = gpool.tile([P, NT], dt.float32)

            # PE warmup: ~20 matmuls on zeroed tiles, independent of any DMA,
            # so the HAM clock-gate is released before the first real matmul.
            warm_w = warmp.tile([P, P], dt.bfloat16)
            warm_x = warmp.tile([P, TN], dt.bfloat16)
            warm_ps = psw.tile([P, TN], dt.float32)
            nc.vector.memset(warm_w[:], 0)
            nc.vector.memset(warm_x[:], 0)
            NWARM = 20
            for i in range(NWARM):
                nc.tensor.matmul(
                    warm_ps[:], warm_w[:], warm_x[:],
                    start=(i == 0), stop=(i == NWARM - 1),
                )

            # sync HWDGE ring: x tiles + w1 (matmul-1 critical path)
            x_tiles_sb = {}
            for i in range(min(XPRE, NTILES)):
                e, t0, tn = tiles[i]
                xt = xpool.tile([P, KO, TN], dt.bfloat16, tag="x", name="x_sb")
                nc.sync.dma_start(xt[:], x.ap()[i])
                x_tiles_sb[i] = xt
            # first expert's w1 split in two ko-halves so matmul 1 starts early
            e0 = expert_order[0]
            nc.sync.dma_start(w1_sb[:, e0, 0 : KO // 2], w1.ap()[:, e0, 0 : KO // 2])
            nc.sync.dma_start(w1_sb[:, e0, KO // 2 :], w1.ap()[:, e0, KO // 2 :])
            for e in expert_order[1:]:
                nc.sync.dma_start(w1_sb[:, e], w1.ap()[:, e])

            # scalar HWDGE ring: gates + w2 + y out
            nc.scalar.dma_start(g_sb[:], g.ap()[:])
            for e in expert_order:
                nc.scalar.dma_start(w2_sb[:, e], w2.ap()[:, e])

            for i, (e, t0, tn) in enumerate(tiles):
                if i in x_tiles_sb:
                    x_sb = x_tiles_sb.pop(i)
                else:
                    x_sb = xpool.tile([P, KO, TN], dt.bfloat16, tag="x", name="x_sb")
                    nc.sync.dma_start(x_sb[:], x.ap()[i])

                # H^T tile [dff_slice, tn] bf16
                h_sb = hpool.tile([P, MS, TN], dt.bfloat16, tag="h", name="h_sb")
                for mf in range(MS):
                    ph = ps1.tile([P, TN], dt.float32, tag="ph", name="ph")
                    for ko in range(KO):
                        nc.tensor.matmul(
                            ph[:, :tn],
                            w1_sb[:, e, ko, mf * P : (mf + 1) * P],
                            x_sb[:, ko, :tn],
                            start=(ko == 0),
                            stop=(ko == KO - 1),
                        )
                    nc.scalar.activation(
                        h_sb[:, mf, :tn],
                        ph[:, :tn],
                        mybir.ActivationFunctionType.Silu,
                    )

                nb = tn // P
                y_sb = ypool.tile([P, TN // P, D_MODEL], dt.bfloat16, tag="y", name="y_sb")
                for mt in range(nb):
                    mtg = t0 // P + mt
                    for nf in range(D_MODEL // 512):
                        py = ps2.tile([P, 512], dt.float32, tag="py", name="py")
                        for mf in range(MS):
                            nc.tensor.matmul(
                                py[:],
                                h_sb[:, mf, mt * P : (mt + 1) * P],
                                w2_sb[:, e, mf, nf * 512 : (nf + 1) * 512],
                                start=(mf == 0),
                                stop=(mf == MS - 1),
                            )
                        nc.vector.tensor_scalar_mul(
                            y_sb[:, mt, nf * 512 : (nf + 1) * 512],
                            py[:],
                            g_sb[:, mtg : mtg + 1],
                        )
                nc.scalar.dma_start(y.ap()[:, t0 // P : t0 // P + nb, :], y_sb[:, :nb, :])

    nc.compile()
    return nc


def _route(xf: np.ndarray, router: np.ndarray):
    """Host-side top-2 routing. Groups (token, slot) pairs by expert, pads
    each expert segment to a multiple of 128."""
    T = xf.shape[0]
    logits = xf @ router  # [T, E] f32
    # top-2 (desc value, ties -> lower index, matching jax.lax.top_k)
    ti = np.argsort(-logits, axis=1, kind="stable")[:, :TOP_K]  # [T, 2]
    tv = np.take_along_axis(logits, ti, axis=1)
    ex = np.exp(tv - tv[:, 0:1])
    w = (ex / ex.sum(axis=1, keepdims=True)).astype(np.float32)  # [T, 2]

    experts_all = ti.T.ravel()  # [2T] slot-major
    gates_all = w.T.ravel()
    tokens_all = np.tile(np.arange(T, dtype=np.int64), TOP_K)

    order = np.argsort(experts_all, kind="stable")
    sorted_experts = experts_all[order]
    counts = np.bincount(sorted_experts, minlength=N_EXPERTS)
    seg_sizes = tuple(int(math.ceil(c / P)) * P for c in counts)
    starts_pad = np.concatenate([[0], np.cumsum(seg_sizes)[:-1]])
    starts = np.concatenate([[0], np.cumsum(counts)[:-1]])
    rank = np.arange(TOP_K * T) - starts[sorted_experts]
    R = int(sum(seg_sizes))

    slot_of_pair = np.empty(TOP_K * T, dtype=np.int64)
    slot_of_pair[order] = starts_pad[sorted_experts] + rank

    slot_token = np.full(R, T, dtype=np.int64)  # T = zero-column sentinel
    slot_token[slot_of_pair] = tokens_all
    slot_gate = np.zeros(R, dtype=np.float32)
    slot_gate[slot_of_pair] = gates_all
    return seg_sizes, slot_token, slot_gate, slot_of_pair


def kernel(x, router, W1, W2, _trace=False):
    from concourse.bass_utils import run_bass_kernel_spmd

    B, S, d = x.shape
    T = B * S
    xf = np.ascontiguousarray(x.reshape(T, d), dtype=np.float32)

    seg_sizes, slot_token, slot_gate, slot_of_pair = _route(xf, np.asarray(router))
    R = int(sum(seg_sizes))
    NT = R // P
    tiles = _token_tiles(seg_sizes)
    NTILES = len(tiles)

    bf16 = ml_dtypes.bfloat16
    # x^T with a trailing zero column as sentinel, partition-major [128, KO, T+1]
    xfT = np.concatenate([xf.T, np.zeros((d, 1), np.float32)], axis=1).astype(bf16)
    xfT = np.ascontiguousarray(xfT.reshape(KO, P, T + 1).transpose(1, 0, 2))
    # pre-tiled x: [NTILES, 128, KO, TN]
    col_idx = np.full((NTILES, TN), T, dtype=np.int64)
    for i, (e, t0, tn) in enumerate(tiles):
        col_idx[i, :tn] = slot_token[t0 : t0 + tn]
    x_dev = np.ascontiguousarray(
        xfT[:, :, col_idx.ravel()]
        .reshape(P, KO, NTILES, TN)
        .transpose(2, 0, 1, 3)
    )

    W1b = np.asarray(W1).astype(bf16)  # [E, 1024, 4096]
    W2b = np.asarray(W2).astype(bf16)  # [E, 4096, 1024]
    g_dev = np.ascontiguousarray(slot_gate.reshape(NT, P).T)  # [128, NT]

    key = seg_sizes
    nc = _KERNEL_CACHE.get(key)
    if nc is None:
        nc = _build_device_kernel(seg_sizes)
        _KERNEL_CACHE[key] = nc

    in_maps = []
    for c in range(N_CORES):
        sl = slice(c * DS, (c + 1) * DS)
        # [128, E, KO, DS]
        w1_c = np.ascontiguousarray(
            W1b[:, :, sl].reshape(N_EXPERTS, KO, P, DS).transpose(2, 0, 1, 3)
        )
        # [128, E, MS, D_MODEL]
        w2_c = np.ascontiguousarray(
            W2b[:, sl, :].reshape(N_EXPERTS, MS, P, D_MODEL).transpose(2, 0, 1, 3)
        )
        in_maps.append({"x": x_dev, "w1": w1_c, "w2": w2_c, "g": g_dev})

    kw = {}
    if _trace:
        kw = {"trace": True, "trace_cores": list(range(N_CORES))}
    res = run_bass_kernel_spmd(nc, in_maps, core_ids=list(range(N_CORES)), **kw)

    # sum the 8 partial outputs in f32; y layout [128, NT, 1024] -> [R, 1024]
    acc = np.zeros((R, D_MODEL), np.float32)
    for c in range(N_CORES):
        yc = res.results[c]["y"]  # [128, NT, 1024] bf16
        acc += yc.transpose(1, 0, 2).reshape(R, D_MODEL).astype(np.float32)

    out = acc[slot_of_pair[:T]] + acc[slot_of_pair[T:]]
    out = out.reshape(B, S, d).astype(np.float32)
    if _trace:
        return out, res
    return out


# revision 7
# speedup vs baseline: 1.0454x; 1.0454x over previous
"""MoE layer (top-2 of 8 experts, d=1024, d_ff=4096) on 8 TRN2 NeuronCores.

Strategy: d_ff-parallel (tensor-parallel over the FFN hidden dim). The
router / top-k / softmax-gate computation is tiny (0.05% of FLOPs) and runs
on host in numpy. Tokens are grouped by expert into a single padded stream of
R rows (each expert segment padded to a multiple of 128). Every core
processes ALL R rows but only a 512-wide slice of d_ff:

    y_c = gate * (swish(x @ W1[e][:, c*512:(c+1)*512]) @ W2[e][c*512:(c+1)*512, :])

so per-core work is exactly 1/8 of the routed FLOPs with zero load
imbalance. The host sums the 8 partial outputs and combines the two expert
contributions per token. All matmuls run in bf16 with fp32 PSUM
accumulation; partial outputs return as bf16 (summed in f32 on host).

All device inputs are host-prearranged so every DMA is per-partition
contiguous (x is pre-tiled into [NTILES, 128, 8, 512] blocks).
"""

import math

import numpy as np
import ml_dtypes

D_MODEL, D_FF, N_EXPERTS, TOP_K = 1024, 4096, 8, 2
N_CORES = 8
P = 128
TN = 512  # token tile (free dim of matmul 1)
DS = D_FF // N_CORES  # 512: per-core d_ff slice
KO = D_MODEL // P  # 8 contraction tiles for matmul 1
MS = DS // P  # 4 d_ff subtiles per core
XPRE = 4  # x tiles prefetched ahead of the weight bulk

_KERNEL_CACHE: dict[tuple, object] = {}


def _token_tiles(seg_sizes):
    """(expert, start, size) tiles, sizes <=TN, multiples of 128, not
    crossing expert-segment boundaries."""
    tiles = []
    t0 = 0
    for e, seg in enumerate(seg_sizes):
        off = 0
        while off < seg:
            tn = min(TN, seg - off)
            tiles.append((e, t0 + off, tn))
            off += tn
        t0 += seg
    return tiles


def _build_device_kernel(seg_sizes: tuple[int, ...]):
    """Per-core program over the padded token stream.

    Inputs (per core):
      x  [NTILES, 128, 8, 512] bf16  pre-tiled x^T: x[i, p, ko, n] =
                                     xf[tok(tile_i, n), ko*128+p]
      w1 [128, 8, 8, 512]   bf16  w1[p, e, ko, f] = W1[e, ko*128+p, c*512+f]
      w2 [128, 8, 4, 1024]  bf16  w2[p, e, m, n] = W2[e, c*512+m*128+p, n]
      g  [128, R//128]      f32   g[p, a] = gate[a*128+p]
    Output:
      y  [128, R//128, 1024] bf16 partial FFN output (gate-scaled)
    """
    import concourse.mybir as mybir
    import concourse.tile as tile
    from concourse import bacc

    dt = mybir.dt
    R = sum(seg_sizes)
    NT = R // P
    tiles = _token_tiles(seg_sizes)
    NTILES = len(tiles)

    nc = bacc.Bacc("TRN2", target_bir_lowering=False, debug=False)

    x = nc.dram_tensor("x", [NTILES, P, KO, TN], dt.bfloat16, kind="ExternalInput")
    w1 = nc.dram_tensor("w1", [P, N_EXPERTS, KO, DS], dt.bfloat16, kind="ExternalInput")
    w2 = nc.dram_tensor("w2", [P, N_EXPERTS, MS, D_MODEL], dt.bfloat16, kind="ExternalInput")
    g = nc.dram_tensor("g", [P, NT], dt.float32, kind="ExternalInput")
    y = nc.dram_tensor("y", [P, NT, D_MODEL], dt.bfloat16, kind="ExternalOutput")

    first_e = tiles[0][0]
    expert_order = [first_e] + [e for e in range(N_EXPERTS) if e != first_e]

    with tile.TileContext(nc) as tc:
        with (
            tc.tile_pool(name="wpool", bufs=1) as wpool,
            tc.tile_pool(name="gpool", bufs=1) as gpool,
            tc.tile_pool(name="warmp", bufs=1) as warmp,
            tc.tile_pool(name="xpool", bufs=XPRE) as xpool,
            tc.tile_pool(name="hpool", bufs=2) as hpool,
            tc.tile_pool(name="ypool", bufs=6) as ypool,
            tc.tile_pool(name="ps1", bufs=3, space="PSUM") as ps1,
            tc.tile_pool(name="ps2", bufs=4, space="PSUM") as ps2,
            tc.tile_pool(name="psw", bufs=1, space="PSUM") as psw,
        ):
            w1_sb = wpool.tile([P, N_EXPERTS, KO, DS], dt.bfloat16)
            w2_sb = wpool.tile([P, N_EXPERTS, MS, D_MODEL], dt.bfloat16)
            g_sb = gpool.tile([P, NT], dt.float32)

            # PE warmup: ~20 matmuls on zeroed tiles, independent of any DMA,
            # so the HAM clock-gate is released before the first real matmul.
            warm_w = warmp.tile([P, P], dt.bfloat16)
            warm_x = warmp.tile([P, TN], dt.bfloat16)
            warm_ps = psw.tile([P, TN], dt.float32)
            nc.vector.memset(warm_w[:], 0)
            nc.vector.memset(warm_x[:], 0)
            NWARM = 20
            for i in range(NWARM):
                nc.tensor.matmul(
                    warm_ps[:], warm_w[:], warm_x[:],
                    start=(i == 0), stop=(i == NWARM - 1),
                )

            # sync HWDGE ring: x tiles only (steady stream, tight deadlines).
            # First tile split in two ko-halves so matmul 1 starts earlier.
            x_tiles_sb = {}
            for i in range(min(XPRE, NTILES)):
                e, t0, tn = tiles[i]
                xt = xpool.tile([P, KO, TN], dt.bfloat16, tag="x", name="x_sb")
                if i == 0:
                    nc.sync.dma_start(xt[:, 0 : KO // 2], x.ap()[i, :, 0 : KO // 2])
                    nc.sync.dma_start(xt[:, KO // 2 :], x.ap()[i, :, KO // 2 :])
                else:
                    nc.sync.dma_start(xt[:], x.ap()[i])
                x_tiles_sb[i] = xt

            # scalar HWDGE ring: gates + weights. First two experts up-front;
            # the rest drip-fed at tile starts (well ahead of their segments).
            def load_expert_weights(e, split_w1=False):
                if split_w1:
                    nc.scalar.dma_start(
                        w1_sb[:, e, 0 : KO // 2], w1.ap()[:, e, 0 : KO // 2]
                    )
                    nc.scalar.dma_start(
                        w1_sb[:, e, KO // 2 :], w1.ap()[:, e, KO // 2 :]
                    )
                else:
                    nc.scalar.dma_start(w1_sb[:, e], w1.ap()[:, e])
                nc.scalar.dma_start(w2_sb[:, e], w2.ap()[:, e])

            nc.scalar.dma_start(g_sb[:], g.ap()[:])
            load_expert_weights(expert_order[0], split_w1=True)
            if len(expert_order) > 1:
                load_expert_weights(expert_order[1])
            pending_experts = list(expert_order[2:])

            for i, (e, t0, tn) in enumerate(tiles):
                if i >= 1 and pending_experts:
                    load_expert_weights(pending_experts.pop(0))
                if i in x_tiles_sb:
                    x_sb = x_tiles_sb.pop(i)
                else:
                    x_sb = xpool.tile([P, KO, TN], dt.bfloat16, tag="x", name="x_sb")
                    nc.sync.dma_start(x_sb[:], x.ap()[i])

                # H^T tile [dff_slice, tn] bf16
                h_sb = hpool.tile([P, MS, TN], dt.bfloat16, tag="h", name="h_sb")
                for mf in range(MS):
                    ph = ps1.tile([P, TN], dt.float32, tag="ph", name="ph")
                    for ko in range(KO):
                        nc.tensor.matmul(
                            ph[:, :tn],
                            w1_sb[:, e, ko, mf * P : (mf + 1) * P],
                            x_sb[:, ko, :tn],
                            start=(ko == 0),
                            stop=(ko == KO - 1),
                        )
                    nc.scalar.activation(
                        h_sb[:, mf, :tn],
                        ph[:, :tn],
                        mybir.ActivationFunctionType.Silu,
                    )

                nb = tn // P
                for mt in range(nb):
                    mtg = t0 // P + mt
                    y_sb = ypool.tile([P, D_MODEL], dt.bfloat16, tag="y", name="y_sb")
                    for nf in range(D_MODEL // 512):
                        py = ps2.tile([P, 512], dt.float32, tag="py", name="py")
                        for mf in range(MS):
                            nc.tensor.matmul(
                                py[:],
                                h_sb[:, mf, mt * P : (mt + 1) * P],
                                w2_sb[:, e, mf, nf * 512 : (nf + 1) * 512],
                                start=(mf == 0),
                                stop=(mf == MS - 1),
                            )
                        nc.vector.tensor_scalar_mul(
                            y_sb[:, nf * 512 : (nf + 1) * 512],
                            py[:],
                            g_sb[:, mtg : mtg + 1],
                        )
                    # y out on the gpsimd SWDGE ring — keeps both HWDGE rings
                    # free for the x and weight streams
                    nc.gpsimd.dma_start(y.ap()[:, mtg, :], y_sb[:])

    nc.compile()
    return nc


def _route(xf: np.ndarray, router: np.ndarray):
    """Host-side top-2 routing. Groups (token, slot) pairs by expert, pads
    each expert segment to a multiple of 128."""
    T = xf.shape[0]
    logits = xf @ router  # [T, E] f32
    # top-2 (desc value, ties -> lower index, matching jax.lax.top_k)
    ti = np.argsort(-logits, axis=1, kind="stable")[:, :TOP_K]  # [T, 2]
    tv = np.take_along_axis(logits, ti, axis=1)
    ex = np.exp(tv - tv[:, 0:1])
    w = (ex / ex.sum(axis=1, keepdims=True)).astype(np.float32)  # [T, 2]

    experts_all = ti.T.ravel()  # [2T] slot-major
    gates_all = w.T.ravel()
    tokens_all = np.tile(np.arange(T, dtype=np.int64), TOP_K)

    order = np.argsort(experts_all, kind="stable")
    sorted_experts = experts_all[order]
    counts = np.bincount(sorted_experts, minlength=N_EXPERTS)
    seg_sizes = tuple(int(math.ceil(c / P)) * P for c in counts)
    starts_pad = np.concatenate([[0], np.cumsum(seg_sizes)[:-1]])
    starts = np.concatenate([[0], np.cumsum(counts)[:-1]])
    rank = np.arange(TOP_K * T) - starts[sorted_experts]
    R = int(sum(seg_sizes))

    slot_of_pair = np.empty(TOP_K * T, dtype=np.int64)
    slot_of_pair[order] = starts_pad[sorted_experts] + rank

    slot_token = np.full(R, T, dtype=np.int64)  # T = zero-column sentinel
    slot_token[slot_of_pair] = tokens_all
    slot_gate = np.zeros(R, dtype=np.float32)
    slot_gate[slot_of_pair] = gates_all
    return seg_sizes, slot_token, slot_gate, slot_of_pair


def kernel(x, router, W1, W2, _trace=False):
    from concourse.bass_utils import run_bass_kernel_spmd

    B, S, d = x.shape
    T = B * S
    xf = np.ascontiguousarray(x.reshape(T, d), dtype=np.float32)

    seg_sizes, slot_token, slot_gate, slot_of_pair = _route(xf, np.asarray(router))
    R = int(sum(seg_sizes))
    NT = R // P
    tiles = _token_tiles(seg_sizes)
    NTILES = len(tiles)

    bf16 = ml_dtypes.bfloat16
    # x^T with a trailing zero column as sentinel, partition-major [128, KO, T+1]
    xfT = np.concatenate([xf.T, np.zeros((d, 1), np.float32)], axis=1).astype(bf16)
    xfT = np.ascontiguousarray(xfT.reshape(KO, P, T + 1).transpose(1, 0, 2))
    # pre-tiled x: [NTILES, 128, KO, TN]
    col_idx = np.full((NTILES, TN), T, dtype=np.int64)
    for i, (e, t0, tn) in enumerate(tiles):
        col_idx[i, :tn] = slot_token[t0 : t0 + tn]
    x_dev = np.ascontiguousarray(
        xfT[:, :, col_idx.ravel()]
        .reshape(P, KO, NTILES, TN)
        .transpose(2, 0, 1, 3)
    )

    W1b = np.asarray(W1).astype(bf16)  # [E, 1024, 4096]
    W2b = np.asarray(W2).astype(bf16)  # [E, 4096, 1024]
    g_dev = np.ascontiguousarray(slot_gate.reshape(NT, P).T)  # [128, NT]

    key = seg_sizes
    nc = _KERNEL_CACHE.get(key)
    if nc is None:
        nc = _build_device_kernel(seg_sizes)
        _KERNEL_CACHE[key] = nc

    in_maps = []
    for c in range(N_CORES):
        sl = slice(c * DS, (c + 1) * DS)
        # [128, E, KO, DS]
        w1_c = np.ascontiguousarray(
            W1b[:, :, sl].reshape(N_EXPERTS, KO, P, DS).transpose(2, 0, 1, 3)
        )
        # [128, E, MS, D_MODEL]
        w2_c = np.ascontiguousarray(
            W2b[:, sl, :].reshape(N_EXPERTS, MS, P, D_MODEL).transpose(2, 0, 1, 3)
        )
        in_maps.append({"x": x_dev, "w1": w1_c, "w2": w2_c, "g": g_dev})

    kw = {}
    if _trace:
        kw = {"trace": True, "trace_cores": list(range(N_CORES))}
    res = run_bass_kernel_spmd(nc, in_maps, core_ids=list(range(N_CORES)), **kw)

    # sum the 8 partial outputs in f32; y layout [128, NT, 1024] -> [R, 1024]
    acc = np.zeros((R, D_MODEL), np.float32)
    for c in range(N_CORES):
        yc = res.results[c]["y"]  # [128, NT, 1024] bf16
        acc += yc.transpose(1, 0, 2).reshape(R, D_MODEL).astype(np.float32)

    out = acc[slot_of_pair[:T]] + acc[slot_of_pair[T:]]
    out = out.reshape(B, S, d).astype(np.float32)
    if _trace:
        return out, res
    return out
